# revision 2
# baseline (speedup 1.0000x reference)
"""FANeuson Trainium2 kernel, v10.

Same math as v4 (bf16 local-coordinate chain, direct mask outputs, bf16 va)
plus two structural fixes for the in-order engine sequencers:

  * Block-major padded DRAM layouts [128, NB, NL, LB]: every x/va/sp block
    DMA is one contiguous descriptor per partition (the SP sequencer's DMA
    dispatch cost scales with descriptor count).  The T+1 edge planes are
    assembled on the host.
  * Software-pipelined emission: the refractory chain is a 5-instruction
    dependency spine per chunk; the DVE sequencer (wait-queue depth 4)
    stalls inside it.  Independent DVE work -- next block's prep (carry,
    scan, cm) and previous block's outputs (m2, sp, m12, va) -- is emitted
    BETWEEN chain chunks so the sequencer always has ready instructions.

Sharding: batch 16 -> 2 per core across 8 cores.
"""

import numpy as np
from contextlib import ExitStack

import ml_dtypes

import concourse.bass as bass
import concourse.tile as tile
from concourse import bacc, mybir
from concourse.bass_utils import run_bass_kernel_spmd

dt = mybir.dt
Alu = mybir.AluOpType

B, T, F = 16, 4096, 512
NCORES = 8
BL = B // NCORES
G = F // 128
NL = BL * G
CH = 41
LB = 8 * CH                  # 328
NB = (T + LB - 1) // LB      # 13 blocks (last one short: 160)
ALPHA = np.float32(0.001)
BF16 = ml_dtypes.bfloat16


def _mk(a, dims):
    return bass.AP(a.tensor, a.offset, [list(d) for d in dims])


def _bcast_mid(a, n):
    d = [list(x) for x in a.ap]
    assert len(d) == 2, d
    return _mk(a, [d[0], [0, n], d[1]])


def _col_bcast(a, w):
    d = [list(x) for x in a.ap]
    assert len(d) == 3 and d[2][1] == 1, d
    return _mk(a, [d[0], d[1], [0, w]])


def _sq(a):
    d = [list(x) for x in a.ap]
    assert len(d) == 3 and d[2][1] == 1, d
    return _mk(a, [d[0], d[1]])


def alternating_cs(Tt):
    one_m_a = np.float64(1.0) - np.float64(ALPHA)
    c_near = np.float32(one_m_a)
    if np.float64(c_near) > one_m_a:
        c_hi, c_lo = c_near, np.nextafter(c_near, np.float32(0))
    else:
        c_lo, c_hi = c_near, np.nextafter(c_near, np.float32(1))
    cs = np.empty(Tt, np.float32)
    lt = np.log(one_m_a)
    llo, lhi = np.log(np.float64(c_lo)), np.log(np.float64(c_hi))
    acc = 0.0
    for t in range(Tt):
        if abs(acc + llo - (t + 1) * lt) < abs(acc + lhi - (t + 1) * lt):
            cs[t] = c_lo
            acc += llo
        else:
            cs[t] = c_hi
            acc += lhi
    cs[0] = 0.0
    return cs


def _blocks(Tt):
    # two half-size leading blocks fill the cross-engine pipeline faster
    out = [(0, LB // 2), (LB // 2, LB // 2)]
    t0 = LB
    while Tt - t0 > LB:
        out.append((t0, LB))
        t0 += LB
    out.append((t0, Tt - t0))
    return out


PLACEMENT = {"d": "g", "ff": "g", "m2": "v", "sp": "v", "m12": "v", "va": "v"}


def build(Tt=T, reps=1, placement=None):
    pl = dict(PLACEMENT)
    if placement:
        pl.update(placement)

    nc = bacc.Bacc("TRN2", target_bir_lowering=False, debug=False)
    f32 = dt.float32
    bf16 = dt.bfloat16
    csv = alternating_cs(Tt)
    blocks = _blocks(Tt)
    nb = len(blocks)
    nch_of = [(L // CH) + (1 if L % CH else 0) for (_, L) in blocks]

    def eng(key):
        return nc.gpsimd if pl[key] == "g" else nc.vector

    x_d = nc.dram_tensor("x", [128, nb, NL, LB], f32, kind="ExternalInput")
    cs_d = nc.dram_tensor("cs", [128, Tt], f32, kind="ExternalInput")
    p41f_d = nc.dram_tensor("p41f", [128, LB], f32, kind="ExternalInput")
    p41h_d = nc.dram_tensor("p41h", [128, LB], bf16, kind="ExternalInput")
    va_d = nc.dram_tensor("va", [128, nb, NL, LB], bf16, kind="ExternalOutput")
    sp_d = nc.dram_tensor("sp", [128, nb, NL, LB], dt.uint8, kind="ExternalOutput")

    xv = x_d.ap()
    vav = va_d.ap()
    spv = sp_d.ap()

    with tile.TileContext(nc) as tc, ExitStack() as ctx:
        p_const = ctx.enter_context(tc.tile_pool(name="const", bufs=1))
        pools = {}
        for nm, bufs in [
            ("x", 2), ("ax", 2), ("e", 2), ("csr", 2), ("d", 2), ("db", 2),
            ("cm", 2), ("m1", 2), ("ff", 2), ("m2", 2), ("m12", 2),
            ("va", 2), ("sp", 2), ("ck", 2),
        ]:
            pools[nm] = ctx.enter_context(tc.tile_pool(name=nm, bufs=bufs))
        p_st = ctx.enter_context(tc.tile_pool(name="st", bufs=1))

        cs_all = p_const.tile([128, Tt], f32)
        nc.sync.dma_start(cs_all[:], cs_d.ap())
        p41f_t = p_const.tile([128, LB], f32)
        nc.sync.dma_start(p41f_t[:], p41f_d.ap())
        p41h_t = p_const.tile([128, LB], bf16)
        nc.sync.dma_start(p41h_t[:], p41h_d.ap())

        s_blks = [
            p_st.tile([128, NL, n + 1], bf16, tag=f"sb{i}", name=f"sb{i}")
            for i, n in enumerate(nch_of)
        ]
        f_blks = [
            p_st.tile([128, NL, n], bf16, tag=f"fb{i}", name=f"fb{i}")
            for i, n in enumerate(nch_of)
        ]
        nc.vector.memset(s_blks[0][:, :, 0:1], float(CH))

        for rep in range(reps):
            tiles = [dict() for _ in range(nb)]

            def emit_prep(bi, rep=rep, tiles=None):
                """Emit block bi's pre-chain ops; returns DVE thunks to
                interleave into the previous block's chain stream."""
                t0, L = blocks[bi]
                tl = tiles[bi]
                tl["x"] = pools["x"].tile([128, NL, L], f32, tag="x",
                                          name=f"x{bi}_{rep}")
                nc.sync.dma_start(tl["x"][:], xv[:, bi, :, 0:L])
                tl["ax"] = pools["ax"].tile([128, NL, L], f32, tag="ax",
                                            name=f"ax{bi}_{rep}")
                nc.scalar.mul(tl["ax"][:], tl["x"][:], float(ALPHA))
                tl["csr"] = pools["csr"].tile([128, NL, L], f32, tag="csr",
                                              name=f"csr{bi}_{rep}")
                nc.gpsimd.tensor_copy(
                    tl["csr"][:], _bcast_mid(cs_all[:, t0 : t0 + L], NL)
                )

                dve = []
                if bi == 0:
                    dve.append(lambda: nc.vector.tensor_copy(
                        tl["ax"][:, :, 0:1], tl["x"][:, :, 0:1]))
                else:
                    def carry_ops():
                        prev_e = tiles[bi - 1]["e"]
                        Lp = blocks[bi - 1][1]
                        cr = pools["ck"].tile([128, NL], f32, tag="cr",
                                              name=f"cr{bi}_{rep}")
                        nc.vector.tensor_scalar(
                            cr[:], _sq(prev_e[:, :, Lp - 1 :]),
                            float(csv[t0]), None, Alu.mult,
                        )
                        nc.vector.tensor_tensor(
                            _sq(tl["ax"][:, :, 0:1]), _sq(tl["ax"][:, :, 0:1]),
                            cr[:], Alu.add,
                        )
                    dve.append(carry_ops)

                def scan_op():
                    tl["e"] = pools["e"].tile([128, NL, L], f32, tag="e",
                                              name=f"e{bi}_{rep}")
                    ef = _mk(tl["e"][:], [list(tl["e"][:].ap[0]), [1, NL * L]])
                    csf = _mk(tl["csr"][:], [list(tl["csr"][:].ap[0]), [1, NL * L]])
                    axf = _mk(tl["ax"][:], [list(tl["ax"][:].ap[0]), [1, NL * L]])
                    nc.vector.tensor_tensor_scan(
                        ef, csf, axf, 0.0, Alu.mult, Alu.add
                    )
                    # d/qsq/db chase the scan on their own engines
                    tl["d"] = pools["d"].tile([128, NL, L], f32, tag="d",
                                              name=f"d{bi}_{rep}")
                    eng("d").tensor_tensor(tl["d"][:], tl["e"][:], tl["x"][:],
                                           Alu.subtract)
                    tl["q"] = pools["ax"].tile([128, NL, L], f32, tag="ax",
                                               name=f"q{bi}_{rep}")
                    nc.scalar.square(tl["q"][:], tl["d"][:])
                    tl["db"] = pools["db"].tile([128, NL, L], bf16, tag="db",
                                                name=f"db{bi}_{rep}")
                    nc.scalar.copy(tl["db"][:], tl["d"][:])
                dve.append(scan_op)

                def cm_op():
                    tl["cm"] = pools["cm"].tile([128, NL, L], bf16, tag="cm",
                                                name=f"cm{bi}_{rep}")
                    nc.vector.scalar_tensor_tensor(
                        tl["cm"][:], tl["q"][:], 1.0,
                        _bcast_mid(p41f_t[:, :L], NL), Alu.is_ge, Alu.mult,
                    )
                    tl["m1"] = pools["m1"].tile([128, NL, L], bf16, tag="m1",
                                                name=f"m1{bi}_{rep}")
                dve.append(cm_op)
                return dve

            def emit_out(bi, rep=rep, tiles=None):
                """Emit block bi's output ops (needs its chain complete);
                returns DVE thunks."""
                t0, L = blocks[bi]
                tl = tiles[bi]
                nch_f = L // CH
                rem = L % CH
                dve = []

                def ff_op():
                    tl["ff"] = pools["ff"].tile([128, NL, L], bf16, tag="ff",
                                                name=f"ff{bi}_{rep}")
                    parts = [(0, nch_f, CH)] + (
                        [(nch_f * CH, 1, rem)] if rem else []
                    )
                    for lo, nch, w in parts:
                        ffsl = tl["ff"][:, :, lo : lo + nch * w]
                        dims = [list(x) for x in ffsl.ap]
                        st = dims[2][0]
                        ff_perm = _mk(ffsl, [dims[0], dims[1], [st, w],
                                             [st * w, nch]])
                        cl = lo // CH
                        fsl = f_blks[bi][:, :, cl : cl + nch]
                        fdims = [list(x) for x in fsl.ap]
                        f_perm = _mk(fsl, [fdims[0], fdims[1], [0, w], fdims[2]])
                        eng("ff").tensor_copy(ff_perm, f_perm)

                def m2_op():
                    tl["m2"] = pools["m2"].tile([128, NL, L], bf16, tag="m2",
                                                name=f"m2{bi}_{rep}")
                    eng("m2").tensor_tensor(
                        tl["m2"][:], _bcast_mid(p41h_t[:, :L], NL), tl["ff"][:],
                        Alu.is_gt,
                    )

                def sp_op():
                    tl["spb"] = pools["cm"].tile([128, NL, L], bf16, tag="cm",
                                                 name=f"spb{bi}_{rep}")
                    eng("sp").tensor_tensor(
                        tl["spb"][:], _bcast_mid(p41h_t[:, :L], NL), tl["ff"][:],
                        Alu.is_equal,
                    )
                    tl["sp"] = pools["sp"].tile([128, NL, L], dt.uint8,
                                                tag="sp", name=f"sp{bi}_{rep}")
                    nc.scalar.copy(tl["sp"][:], tl["spb"][:])
                    nc.gpsimd.dma_start(spv[:, bi, :, 0:L], tl["sp"][:])

                def m12_op():
                    tl["m12"] = pools["m12"].tile([128, NL, L], bf16,
                                                  tag="m12", name=f"m12{bi}_{rep}")
                    nc.vector.tensor_tensor(tl["m12"][:], tl["m1"][:],
                                            tl["m2"][:], Alu.mult)

                def va_op():
                    tl["va"] = pools["va"].tile([128, NL, L], bf16, tag="va",
                                                name=f"va{bi}_{rep}")
                    nc.vector.tensor_tensor(tl["va"][:], tl["m12"][:],
                                            tl["db"][:], Alu.mult)
                    nc.scalar.dma_start(vav[:, bi, :, 0:L], tl["va"][:])

                ff_op()  # pool op; emit immediately
                dve.extend([m2_op, sp_op, m12_op, va_op])
                return dve

            def emit_chain(bi, fillers, rep=rep, tiles=None):
                t0, L = blocks[bi]
                tl = tiles[bi]
                nch_f = L // CH
                rem = L % CH
                widths = [CH] * nch_f + ([rem] if rem else [])
                nch_b = len(widths)
                fi = 0

                def fill(n=1):
                    nonlocal fi
                    for _ in range(n):
                        if fi < len(fillers):
                            fillers[fi]()
                            fi += 1

                for ci, w in enumerate(widths):
                    cg = t0 // CH + ci
                    lo = ci * CH
                    nc.vector.tensor_tensor(
                        tl["m1"][:, :, lo : lo + w],
                        _bcast_mid(p41h_t[:, lo : lo + w], NL),
                        _col_bcast(s_blks[bi][:, :, ci : ci + 1], w),
                        Alu.is_le,
                    )
                    z_t = pools["ck"].tile([128, NL, CH], bf16, tag="Z",
                                           name=f"z{cg}_{rep}")
                    nc.vector.tensor_tensor(
                        z_t[:, :, :w], tl["m1"][:, :, lo : lo + w],
                        tl["cm"][:, :, lo : lo + w], Alu.mult,
                    )
                    nc.vector.tensor_reduce(
                        _sq(f_blks[bi][:, :, ci : ci + 1]), z_t[:, :, :w],
                        mybir.AxisListType.X, Alu.max,
                    )
                    fill()
                    if ci < nch_b - 1:
                        nxt = s_blks[bi][:, :, ci + 1 : ci + 2]
                    elif bi + 1 < nb:
                        nxt = s_blks[bi + 1][:, :, 0:1]
                    else:
                        nxt = None
                    if nxt is not None:
                        fcol = _sq(f_blks[bi][:, :, ci : ci + 1])
                        h_t = pools["ck"].tile([128, NL], bf16, tag="h",
                                               name=f"h{cg}_{rep}")
                        nc.vector.tensor_scalar(
                            h_t[:], fcol, 1.0, float(CH), Alu.is_lt, Alu.mult
                        )
                        nc.vector.tensor_tensor(_sq(nxt), fcol, h_t[:], Alu.add)
                    fill()
                fill(len(fillers))  # flush

            # software pipeline: prep(0); then for each bi: chain(bi)
            # interleaved with prep(bi+1) + out(bi-1); finally out(nb-1).
            pending = emit_prep(0, tiles=tiles)
            for f in pending:
                f()
            for bi in range(nb):
                fillers = []
                if bi + 1 < nb:
                    fillers.extend(emit_prep(bi + 1, tiles=tiles))
                if bi - 1 >= 0:
                    fillers.extend(emit_out(bi - 1, tiles=tiles))
                emit_chain(bi, fillers, tiles=tiles)
            for f in emit_out(nb - 1, tiles=tiles):
                f()

    nc.compile()
    return nc


def host_inputs(x_core, Tt=T):
    csr = alternating_cs(Tt).copy()
    for t0, _ in _blocks(Tt):
        csr[t0] = 0.0
    cs = np.ascontiguousarray(np.broadcast_to(csr, (128, Tt)))
    p41 = (CH - (np.arange(LB) % CH)).astype(np.float32)
    p41f = np.ascontiguousarray(np.broadcast_to(p41, (128, LB)))
    p41h = np.ascontiguousarray(p41f.astype(BF16))
    xr = np.ascontiguousarray(
        x_core.reshape(BL, Tt, G, 128).transpose(3, 0, 2, 1), np.float32
    ).reshape(128, NL, Tt)
    nb = len(_blocks(Tt))
    xp = np.zeros((128, nb, NL, LB), np.float32)
    for bi, (t0, L) in enumerate(_blocks(Tt)):
        xp[:, bi, :, 0:L] = xr[:, :, t0 : t0 + L]
    return {"x": xp, "cs": cs, "p41f": p41f, "p41h": p41h}


def _assemble(blk, Tt, np_dtype, off):
    """[128, nb, NL, LB] block-major -> [128, NL, Tt+1] linear at offset."""
    out = np.zeros((128, NL, Tt + 1), np_dtype)
    for bi, (t0, L) in enumerate(_blocks(Tt)):
        out[:, :, off + t0 : off + t0 + L] = blk[:, bi, :, 0:L]
    return out


_NC = None
LAST_EXEC_NS = None
LAST_RESULT = None


def kernel(input_current, vb_t=None, A_t=None, th_t=None, gain_t=None, tref_t=None):
    global _NC, LAST_EXEC_NS, LAST_RESULT
    x = np.ascontiguousarray(np.asarray(input_current), np.float32)
    assert x.shape == (B, T, F), x.shape
    if _NC is None:
        _NC = build(T)
    in_maps = [host_inputs(x[k * BL : (k + 1) * BL]) for k in range(NCORES)]
    res = run_bass_kernel_spmd(_NC, in_maps, core_ids=list(range(NCORES)))
    LAST_EXEC_NS = res.exec_time_ns
    LAST_RESULT = res

    def untr(a3):
        p, nl, tt = a3.shape
        return a3.reshape(p, BL, G, tt).transpose(1, 3, 2, 0).reshape(
            BL, tt, G * p
        )

    vas, sps = [], []
    for k in range(NCORES):
        vab = _assemble(res.results[k]["va"].astype(np.float32), T, np.float32, 1)
        vab[:, :, 0] = 0.0
        spb = _assemble(res.results[k]["sp"], T, np.uint8, 0)
        spb[:, :, T] = spb[:, :, T - 1]
        vas.append(untr(vab))
        sps.append(untr(spb))
    va = np.concatenate(vas, axis=0)
    sp = np.concatenate(sps, axis=0)
    return va, sp.astype(bool)


# revision 3
# speedup vs baseline: 1.0150x; 1.0150x over previous
"""FANeuson Trainium2 kernel, v14.

Same math as v4 (bf16 local-coordinate chain, direct mask outputs, bf16 va)
plus two structural fixes for the in-order engine sequencers:

  * Block-major padded DRAM layouts [128, NB, NL, LB]: every x/va/sp block
    DMA is one contiguous descriptor per partition (the SP sequencer's DMA
    dispatch cost scales with descriptor count).  The T+1 edge planes are
    assembled on the host.
  * Software-pipelined emission: the refractory chain is a 5-instruction
    dependency spine per chunk; the DVE sequencer (wait-queue depth 4)
    stalls inside it.  Independent DVE work -- next block's prep (carry,
    scan, cm) and previous block's outputs (m2, sp, m12, va) -- is emitted
    BETWEEN chain chunks so the sequencer always has ready instructions.

Sharding: batch 16 -> 2 per core across 8 cores.
"""

import numpy as np
from contextlib import ExitStack

import ml_dtypes

import concourse.bass as bass
import concourse.tile as tile
from concourse import bacc, mybir
from concourse.bass_utils import run_bass_kernel_spmd

dt = mybir.dt
Alu = mybir.AluOpType

B, T, F = 16, 4096, 512
NCORES = 8
BL = B // NCORES
G = F // 128
NL = BL * G
CH = 41
LB = 8 * CH                  # 328
NB = (T + LB - 1) // LB      # 13 blocks (last one short: 160)
ALPHA = np.float32(0.001)
BF16 = ml_dtypes.bfloat16


def _mk(a, dims):
    return bass.AP(a.tensor, a.offset, [list(d) for d in dims])


def _bcast_mid(a, n):
    d = [list(x) for x in a.ap]
    assert len(d) == 2, d
    return _mk(a, [d[0], [0, n], d[1]])


def _col_bcast(a, w):
    d = [list(x) for x in a.ap]
    assert len(d) == 3 and d[2][1] == 1, d
    return _mk(a, [d[0], d[1], [0, w]])


def _sq(a):
    d = [list(x) for x in a.ap]
    assert len(d) == 3 and d[2][1] == 1, d
    return _mk(a, [d[0], d[1]])


def alternating_cs(Tt):
    one_m_a = np.float64(1.0) - np.float64(ALPHA)
    c_near = np.float32(one_m_a)
    if np.float64(c_near) > one_m_a:
        c_hi, c_lo = c_near, np.nextafter(c_near, np.float32(0))
    else:
        c_lo, c_hi = c_near, np.nextafter(c_near, np.float32(1))
    cs = np.empty(Tt, np.float32)
    lt = np.log(one_m_a)
    llo, lhi = np.log(np.float64(c_lo)), np.log(np.float64(c_hi))
    acc = 0.0
    for t in range(Tt):
        if abs(acc + llo - (t + 1) * lt) < abs(acc + lhi - (t + 1) * lt):
            cs[t] = c_lo
            acc += llo
        else:
            cs[t] = c_hi
            acc += lhi
    cs[0] = 0.0
    return cs


def _blocks(Tt):
    # two half-size leading blocks fill the cross-engine pipeline faster
    out = [(0, LB // 2), (LB // 2, LB // 2)]
    t0 = LB
    while Tt - t0 > LB:
        out.append((t0, LB))
        t0 += LB
    out.append((t0, Tt - t0))
    return out


PLACEMENT = {"d": "g", "ff": "g", "m2": "v", "sp": "v", "m12": "v", "va": "v"}


def build(Tt=T, reps=1, placement=None):
    pl = dict(PLACEMENT)
    if placement:
        pl.update(placement)

    nc = bacc.Bacc("TRN2", target_bir_lowering=False, debug=False)
    f32 = dt.float32
    bf16 = dt.bfloat16
    csv = alternating_cs(Tt)
    blocks = _blocks(Tt)
    nb = len(blocks)
    nch_of = [(L // CH) + (1 if L % CH else 0) for (_, L) in blocks]

    def eng(key):
        return nc.gpsimd if pl[key] == "g" else nc.vector

    x_d = nc.dram_tensor("x", [128, nb, NL, LB], f32, kind="ExternalInput")
    cs_d = nc.dram_tensor("cs", [128, Tt], f32, kind="ExternalInput")
    p41f_d = nc.dram_tensor("p41f", [128, LB], f32, kind="ExternalInput")
    p41h_d = nc.dram_tensor("p41h", [128, LB], bf16, kind="ExternalInput")
    va_d = nc.dram_tensor("va", [128, nb, NL, LB], bf16, kind="ExternalOutput")
    sp_d = nc.dram_tensor("sp", [128, nb, NL, LB], dt.uint8, kind="ExternalOutput")

    xv = x_d.ap()
    vav = va_d.ap()
    spv = sp_d.ap()

    with tile.TileContext(nc) as tc, ExitStack() as ctx:
        p_const = ctx.enter_context(tc.tile_pool(name="const", bufs=1))
        pools = {}
        for nm, bufs in [
            ("x", 2), ("ax", 2), ("e", 2), ("csr", 2), ("d", 2), ("db", 2),
            ("cm", 2), ("m1", 2), ("ff", 2), ("m2", 2), ("m12", 2),
            ("va", 2), ("sp", 2), ("ck", 2),
        ]:
            pools[nm] = ctx.enter_context(tc.tile_pool(name=nm, bufs=bufs))
        p_st = ctx.enter_context(tc.tile_pool(name="st", bufs=1))

        cs_all = p_const.tile([128, Tt], f32)
        nc.sync.dma_start(cs_all[:], cs_d.ap())
        p41f_t = p_const.tile([128, LB], f32)
        nc.sync.dma_start(p41f_t[:], p41f_d.ap())
        p41h_t = p_const.tile([128, LB], bf16)
        nc.sync.dma_start(p41h_t[:], p41h_d.ap())

        s_blks = [
            p_st.tile([128, NL, n + 1], bf16, tag=f"sb{i}", name=f"sb{i}")
            for i, n in enumerate(nch_of)
        ]
        f_blks = [
            p_st.tile([128, NL, n], bf16, tag=f"fb{i}", name=f"fb{i}")
            for i, n in enumerate(nch_of)
        ]
        nc.vector.memset(s_blks[0][:, :, 0:1], float(CH))

        for rep in range(reps):
            tiles = [dict() for _ in range(nb)]

            def emit_prep(bi, rep=rep, tiles=None):
                """Emit block bi's pre-chain ops; returns DVE thunks to
                interleave into the previous block's chain stream."""
                t0, L = blocks[bi]
                tl = tiles[bi]
                tl["x"] = pools["x"].tile([128, NL, L], f32, tag="x",
                                          name=f"x{bi}_{rep}")
                nc.sync.dma_start(tl["x"][:], xv[:, bi, :, 0:L])
                tl["ax"] = pools["ax"].tile([128, NL, L], f32, tag="ax",
                                            name=f"ax{bi}_{rep}")
                nc.scalar.mul(tl["ax"][:], tl["x"][:], float(ALPHA))
                tl["csr"] = pools["csr"].tile([128, NL, L], f32, tag="csr",
                                              name=f"csr{bi}_{rep}")
                nc.gpsimd.tensor_copy(
                    tl["csr"][:], _bcast_mid(cs_all[:, t0 : t0 + L], NL)
                )

                dve = []
                if bi == 0:
                    dve.append(lambda: nc.vector.tensor_copy(
                        tl["ax"][:, :, 0:1], tl["x"][:, :, 0:1]))
                else:
                    def carry_ops():
                        prev_e = tiles[bi - 1]["e"]
                        Lp = blocks[bi - 1][1]
                        cr = pools["ck"].tile([128, NL], f32, tag="cr",
                                              name=f"cr{bi}_{rep}")
                        nc.vector.tensor_scalar(
                            cr[:], _sq(prev_e[:, :, Lp - 1 :]),
                            float(csv[t0]), None, Alu.mult,
                        )
                        nc.vector.tensor_tensor(
                            _sq(tl["ax"][:, :, 0:1]), _sq(tl["ax"][:, :, 0:1]),
                            cr[:], Alu.add,
                        )
                    dve.append(carry_ops)

                def scan_op():
                    tl["e"] = pools["e"].tile([128, NL, L], f32, tag="e",
                                              name=f"e{bi}_{rep}")
                    ef = _mk(tl["e"][:], [list(tl["e"][:].ap[0]), [1, NL * L]])
                    csf = _mk(tl["csr"][:], [list(tl["csr"][:].ap[0]), [1, NL * L]])
                    axf = _mk(tl["ax"][:], [list(tl["ax"][:].ap[0]), [1, NL * L]])
                    nc.vector.tensor_tensor_scan(
                        ef, csf, axf, 0.0, Alu.mult, Alu.add
                    )
                    # first chunk of d/q/cm on the fast path (DVE/Act) so the
                    # next chain can start while Pool produces the rest
                    tl["d"] = pools["d"].tile([128, NL, L], f32, tag="d",
                                              name=f"d{bi}_{rep}")
                    tl["q"] = pools["ax"].tile([128, NL, L], f32, tag="ax",
                                               name=f"q{bi}_{rep}")
                    tl["cm"] = pools["cm"].tile([128, NL, L], bf16, tag="cm",
                                                name=f"cm{bi}_{rep}")
                    w0 = min(CH, L)
                    nc.vector.tensor_tensor(
                        tl["d"][:, :, 0:w0], tl["e"][:, :, 0:w0],
                        tl["x"][:, :, 0:w0], Alu.subtract,
                    )
                    nc.scalar.square(tl["q"][:, :, 0:w0], tl["d"][:, :, 0:w0])
                    nc.vector.scalar_tensor_tensor(
                        tl["cm"][:, :, 0:w0], tl["q"][:, :, 0:w0], 1.0,
                        _bcast_mid(p41f_t[:, 0:w0], NL), Alu.is_ge, Alu.mult,
                    )
                    if L > w0:
                        eng("d").tensor_tensor(
                            tl["d"][:, :, w0:], tl["e"][:, :, w0:],
                            tl["x"][:, :, w0:], Alu.subtract,
                        )
                        nc.scalar.square(tl["q"][:, :, w0:], tl["d"][:, :, w0:])
                    tl["db"] = pools["db"].tile([128, NL, L], bf16, tag="db",
                                                name=f"db{bi}_{rep}")
                    nc.scalar.copy(tl["db"][:], tl["d"][:])
                dve.append(scan_op)

                def cm_op():
                    L_ = blocks[bi][1]
                    w0 = min(CH, L_)
                    if L_ > w0:
                        nc.vector.scalar_tensor_tensor(
                            tl["cm"][:, :, w0:], tl["q"][:, :, w0:], 1.0,
                            _bcast_mid(p41f_t[:, w0:L_], NL), Alu.is_ge, Alu.mult,
                        )
                    tl["m1"] = pools["m1"].tile([128, NL, L], bf16, tag="m1",
                                                name=f"m1{bi}_{rep}")
                dve.append(cm_op)
                return dve

            def emit_out(bi, rep=rep, tiles=None):
                """Emit block bi's output ops (needs its chain complete);
                returns DVE thunks."""
                t0, L = blocks[bi]
                tl = tiles[bi]
                nch_f = L // CH
                rem = L % CH
                dve = []

                def ff_op():
                    tl["ff"] = pools["ff"].tile([128, NL, L], bf16, tag="ff",
                                                name=f"ff{bi}_{rep}")
                    parts = [(0, nch_f, CH)] + (
                        [(nch_f * CH, 1, rem)] if rem else []
                    )
                    for lo, nch, w in parts:
                        ffsl = tl["ff"][:, :, lo : lo + nch * w]
                        dims = [list(x) for x in ffsl.ap]
                        st = dims[2][0]
                        ff_perm = _mk(ffsl, [dims[0], dims[1], [st, w],
                                             [st * w, nch]])
                        cl = lo // CH
                        fsl = f_blks[bi][:, :, cl : cl + nch]
                        fdims = [list(x) for x in fsl.ap]
                        f_perm = _mk(fsl, [fdims[0], fdims[1], [0, w], fdims[2]])
                        eng("ff").tensor_copy(ff_perm, f_perm)

                def m2_op():
                    tl["m2"] = pools["m2"].tile([128, NL, L], bf16, tag="m2",
                                                name=f"m2{bi}_{rep}")
                    eng("m2").tensor_tensor(
                        tl["m2"][:], _bcast_mid(p41h_t[:, :L], NL), tl["ff"][:],
                        Alu.is_gt,
                    )

                def sp_op():
                    tl["spb"] = pools["cm"].tile([128, NL, L], bf16, tag="cm",
                                                 name=f"spb{bi}_{rep}")
                    eng("sp").tensor_tensor(
                        tl["spb"][:], _bcast_mid(p41h_t[:, :L], NL), tl["ff"][:],
                        Alu.is_equal,
                    )
                    tl["sp"] = pools["sp"].tile([128, NL, L], dt.uint8,
                                                tag="sp", name=f"sp{bi}_{rep}")
                    nc.scalar.copy(tl["sp"][:], tl["spb"][:])
                    nc.gpsimd.dma_start(spv[:, bi, :, 0:L], tl["sp"][:])

                def m12_op():
                    tl["m12"] = pools["m12"].tile([128, NL, L], bf16,
                                                  tag="m12", name=f"m12{bi}_{rep}")
                    nc.vector.tensor_tensor(tl["m12"][:], tl["m1"][:],
                                            tl["m2"][:], Alu.mult)

                def va_op():
                    tl["va"] = pools["va"].tile([128, NL, L], bf16, tag="va",
                                                name=f"va{bi}_{rep}")
                    nc.vector.tensor_tensor(tl["va"][:], tl["m12"][:],
                                            tl["db"][:], Alu.mult)
                    nc.scalar.dma_start(vav[:, bi, :, 0:L], tl["va"][:])

                ff_op()  # pool op; emit immediately
                dve.extend([m2_op, sp_op, m12_op, va_op])
                return dve

            def emit_chain(bi, fillers, rep=rep, tiles=None):
                t0, L = blocks[bi]
                tl = tiles[bi]
                nch_f = L // CH
                rem = L % CH
                widths = [CH] * nch_f + ([rem] if rem else [])
                nch_b = len(widths)
                fi = 0

                def fill(n=1):
                    nonlocal fi
                    for _ in range(n):
                        if fi < len(fillers):
                            fillers[fi]()
                            fi += 1

                for ci, w in enumerate(widths):
                    cg = t0 // CH + ci
                    lo = ci * CH
                    nc.vector.tensor_tensor(
                        tl["m1"][:, :, lo : lo + w],
                        _bcast_mid(p41h_t[:, lo : lo + w], NL),
                        _col_bcast(s_blks[bi][:, :, ci : ci + 1], w),
                        Alu.is_le,
                    )
                    z_t = pools["ck"].tile([128, NL, CH], bf16, tag="Z",
                                           name=f"z{cg}_{rep}")
                    nc.vector.tensor_tensor(
                        z_t[:, :, :w], tl["m1"][:, :, lo : lo + w],
                        tl["cm"][:, :, lo : lo + w], Alu.mult,
                    )
                    nc.vector.tensor_reduce(
                        _sq(f_blks[bi][:, :, ci : ci + 1]), z_t[:, :, :w],
                        mybir.AxisListType.X, Alu.max,
                    )
                    fill()
                    if ci < nch_b - 1:
                        nxt = s_blks[bi][:, :, ci + 1 : ci + 2]
                    elif bi + 1 < nb:
                        nxt = s_blks[bi + 1][:, :, 0:1]
                    else:
                        nxt = None
                    if nxt is not None:
                        fcol = _sq(f_blks[bi][:, :, ci : ci + 1])
                        h_t = pools["ck"].tile([128, NL], bf16, tag="h",
                                               name=f"h{cg}_{rep}")
                        nc.vector.tensor_scalar(
                            h_t[:], fcol, 1.0, float(CH), Alu.is_lt, Alu.mult
                        )
                        nc.vector.tensor_tensor(_sq(nxt), fcol, h_t[:], Alu.add)
                    fill()
                fill(len(fillers))  # flush

            # software pipeline: prep(0); then for each bi: chain(bi)
            # interleaved with prep(bi+1) + out(bi-1); finally out(nb-1).
            pending = emit_prep(0, tiles=tiles)
            for f in pending:
                f()
            for bi in range(nb):
                fillers = []
                if bi + 1 < nb:
                    fillers.extend(emit_prep(bi + 1, tiles=tiles))
                if bi - 1 >= 0:
                    fillers.extend(emit_out(bi - 1, tiles=tiles))
                emit_chain(bi, fillers, tiles=tiles)
            for f in emit_out(nb - 1, tiles=tiles):
                f()

    nc.compile()
    return nc


def host_inputs(x_core, Tt=T):
    csr = alternating_cs(Tt).copy()
    for t0, _ in _blocks(Tt):
        csr[t0] = 0.0
    cs = np.ascontiguousarray(np.broadcast_to(csr, (128, Tt)))
    p41 = (CH - (np.arange(LB) % CH)).astype(np.float32)
    p41f = np.ascontiguousarray(np.broadcast_to(p41, (128, LB)))
    p41h = np.ascontiguousarray(p41f.astype(BF16))
    xr = np.ascontiguousarray(
        x_core.reshape(BL, Tt, G, 128).transpose(3, 0, 2, 1), np.float32
    ).reshape(128, NL, Tt)
    nb = len(_blocks(Tt))
    xp = np.zeros((128, nb, NL, LB), np.float32)
    for bi, (t0, L) in enumerate(_blocks(Tt)):
        xp[:, bi, :, 0:L] = xr[:, :, t0 : t0 + L]
    return {"x": xp, "cs": cs, "p41f": p41f, "p41h": p41h}


def _assemble(blk, Tt, np_dtype, off):
    """[128, nb, NL, LB] block-major -> [128, NL, Tt+1] linear at offset."""
    out = np.zeros((128, NL, Tt + 1), np_dtype)
    for bi, (t0, L) in enumerate(_blocks(Tt)):
        out[:, :, off + t0 : off + t0 + L] = blk[:, bi, :, 0:L]
    return out


_NC = None
LAST_EXEC_NS = None
LAST_RESULT = None


def kernel(input_current, vb_t=None, A_t=None, th_t=None, gain_t=None, tref_t=None):
    global _NC, LAST_EXEC_NS, LAST_RESULT
    x = np.ascontiguousarray(np.asarray(input_current), np.float32)
    assert x.shape == (B, T, F), x.shape
    if _NC is None:
        _NC = build(T)
    in_maps = [host_inputs(x[k * BL : (k + 1) * BL]) for k in range(NCORES)]
    res = run_bass_kernel_spmd(_NC, in_maps, core_ids=list(range(NCORES)))
    LAST_EXEC_NS = res.exec_time_ns
    LAST_RESULT = res

    def untr(a3):
        p, nl, tt = a3.shape
        return a3.reshape(p, BL, G, tt).transpose(1, 3, 2, 0).reshape(
            BL, tt, G * p
        )

    vas, sps = [], []
    for k in range(NCORES):
        vab = _assemble(res.results[k]["va"].astype(np.float32), T, np.float32, 1)
        vab[:, :, 0] = 0.0
        spb = _assemble(res.results[k]["sp"], T, np.uint8, 0)
        spb[:, :, T] = spb[:, :, T - 1]
        vas.append(untr(vab))
        sps.append(untr(spb))
    va = np.concatenate(vas, axis=0)
    sp = np.concatenate(sps, axis=0)
    return va, sp.astype(bool)


# revision 4
# speedup vs baseline: 1.0290x; 1.0139x over previous
"""FANeuson Trainium2 kernel, v21.

Same math as v4 (bf16 local-coordinate chain, direct mask outputs, bf16 va)
plus two structural fixes for the in-order engine sequencers:

  * Block-major padded DRAM layouts [128, NB, NL, LB]: every x/va/sp block
    DMA is one contiguous descriptor per partition (the SP sequencer's DMA
    dispatch cost scales with descriptor count).  The T+1 edge planes are
    assembled on the host.
  * Software-pipelined emission: the refractory chain is a 5-instruction
    dependency spine per chunk; the DVE sequencer (wait-queue depth 4)
    stalls inside it.  Independent DVE work -- next block's prep (carry,
    scan, cm) and previous block's outputs (m2, sp, m12, va) -- is emitted
    BETWEEN chain chunks so the sequencer always has ready instructions.

Sharding: batch 16 -> 2 per core across 8 cores.
"""

import numpy as np
from contextlib import ExitStack

import ml_dtypes

import concourse.bass as bass
import concourse.tile as tile
from concourse import bacc, mybir
from concourse.bass_utils import run_bass_kernel_spmd

dt = mybir.dt
Alu = mybir.AluOpType

B, T, F = 16, 4096, 512
NCORES = 8
BL = B // NCORES
G = F // 128
NL = BL * G
CH = 41
LB = 8 * CH                  # 328
NB = (T + LB - 1) // LB      # 13 blocks (last one short: 160)
ALPHA = np.float32(0.001)
BF16 = ml_dtypes.bfloat16


def _mk(a, dims):
    return bass.AP(a.tensor, a.offset, [list(d) for d in dims])


def _bcast_mid(a, n):
    d = [list(x) for x in a.ap]
    assert len(d) == 2, d
    return _mk(a, [d[0], [0, n], d[1]])


def _col_bcast(a, w):
    d = [list(x) for x in a.ap]
    assert len(d) == 3 and d[2][1] == 1, d
    return _mk(a, [d[0], d[1], [0, w]])


def _sq(a):
    d = [list(x) for x in a.ap]
    assert len(d) == 3 and d[2][1] == 1, d
    return _mk(a, [d[0], d[1]])


def alternating_cs(Tt):
    one_m_a = np.float64(1.0) - np.float64(ALPHA)
    c_near = np.float32(one_m_a)
    if np.float64(c_near) > one_m_a:
        c_hi, c_lo = c_near, np.nextafter(c_near, np.float32(0))
    else:
        c_lo, c_hi = c_near, np.nextafter(c_near, np.float32(1))
    cs = np.empty(Tt, np.float32)
    lt = np.log(one_m_a)
    llo, lhi = np.log(np.float64(c_lo)), np.log(np.float64(c_hi))
    acc = 0.0
    for t in range(Tt):
        if abs(acc + llo - (t + 1) * lt) < abs(acc + lhi - (t + 1) * lt):
            cs[t] = c_lo
            acc += llo
        else:
            cs[t] = c_hi
            acc += lhi
    cs[0] = 0.0
    return cs


def _blocks(Tt):
    # two half-size leading blocks fill the cross-engine pipeline faster
    out = [(0, LB // 2), (LB // 2, LB // 2)]
    t0 = LB
    while Tt - t0 > LB:
        out.append((t0, LB))
        t0 += LB
    out.append((t0, Tt - t0))
    return out


PLACEMENT = {"d": "g", "ff": "g", "m2": "v", "sp": "v", "m12": "v", "va": "v"}


def build(Tt=T, reps=1, placement=None):
    pl = dict(PLACEMENT)
    if placement:
        pl.update(placement)

    nc = bacc.Bacc("TRN2", target_bir_lowering=False, debug=False)
    f32 = dt.float32
    bf16 = dt.bfloat16
    csv = alternating_cs(Tt)
    blocks = _blocks(Tt)
    nb = len(blocks)
    nch_of = [(L // CH) + (1 if L % CH else 0) for (_, L) in blocks]

    def eng(key):
        return nc.gpsimd if pl[key] == "g" else nc.vector

    x_d = nc.dram_tensor("x", [128, nb, NL, LB], f32, kind="ExternalInput")
    cs_d = nc.dram_tensor("cs", [128, Tt], f32, kind="ExternalInput")
    p41f_d = nc.dram_tensor("p41f", [128, LB], f32, kind="ExternalInput")
    p41h_d = nc.dram_tensor("p41h", [128, LB], bf16, kind="ExternalInput")
    va_d = nc.dram_tensor("va", [128, nb, NL, LB], bf16, kind="ExternalOutput")
    sp_d = nc.dram_tensor("sp", [128, nb, NL, LB], dt.uint8, kind="ExternalOutput")

    xv = x_d.ap()
    vav = va_d.ap()
    spv = sp_d.ap()

    with tile.TileContext(nc) as tc, ExitStack() as ctx:
        p_const = ctx.enter_context(tc.tile_pool(name="const", bufs=1))
        pools = {}
        for nm, bufs in [
            ("x", 2), ("ax", 2), ("e", 2), ("csr", 2), ("d", 2), ("db", 2),
            ("cm", 2), ("m1", 2), ("ff", 2), ("m2", 2), ("m12", 2),
            ("va", 2), ("sp", 2), ("ck", 2),
        ]:
            pools[nm] = ctx.enter_context(tc.tile_pool(name=nm, bufs=bufs))
        p_st = ctx.enter_context(tc.tile_pool(name="st", bufs=1))

        cs_all = p_const.tile([128, Tt], f32)
        # only the first blocks' coefficients up front; the rest streams
        # between the early x DMAs so block 0's input is not stuck behind
        # a 2 MB constant transfer on the SP sequencer
        nc.sync.dma_start(cs_all[:, 0:328], cs_d.ap()[:, 0:328])
        p41f_t = p_const.tile([128, LB], f32)
        nc.sync.dma_start(p41f_t[:], p41f_d.ap())
        p41h_t = p_const.tile([128, LB], bf16)
        nc.sync.dma_start(p41h_t[:], p41h_d.ap())

        s_blks = [
            p_st.tile([128, NL, n + 1], bf16, tag=f"sb{i}", name=f"sb{i}")
            for i, n in enumerate(nch_of)
        ]
        f_blks = [
            p_st.tile([128, NL, n], bf16, tag=f"fb{i}", name=f"fb{i}")
            for i, n in enumerate(nch_of)
        ]
        nc.vector.memset(s_blks[0][:, :, 0:1], float(CH))

        for rep in range(reps):
            tiles = [dict() for _ in range(nb)]

            def emit_prep(bi, rep=rep, tiles=None):
                """Emit block bi's pre-chain ops; returns DVE thunks to
                interleave into the previous block's chain stream."""
                t0, L = blocks[bi]
                tl = tiles[bi]
                tl["x"] = pools["x"].tile([128, NL, L], f32, tag="x",
                                          name=f"x{bi}_{rep}")
                nc.sync.dma_start(tl["x"][:], xv[:, bi, :, 0:L])
                tl["ax"] = pools["ax"].tile([128, NL, L], f32, tag="ax",
                                            name=f"ax{bi}_{rep}")
                nc.scalar.mul(tl["ax"][:], tl["x"][:], float(ALPHA))
                tl["csr"] = pools["csr"].tile([128, NL, L], f32, tag="csr",
                                              name=f"csr{bi}_{rep}")
                nc.gpsimd.tensor_copy(
                    tl["csr"][:], _bcast_mid(cs_all[:, t0 : t0 + L], NL)
                )

                dve = []
                if bi == 0:
                    dve.append(lambda: nc.vector.tensor_copy(
                        tl["ax"][:, :, 0:1], tl["x"][:, :, 0:1]))
                else:
                    def carry_ops():
                        prev_e = tiles[bi - 1]["e"]
                        Lp = blocks[bi - 1][1]
                        cr = pools["ck"].tile([128, NL], f32, tag="cr",
                                              name=f"cr{bi}_{rep}")
                        nc.vector.tensor_scalar(
                            cr[:], _sq(prev_e[:, :, Lp - 1 :]),
                            float(csv[t0]), None, Alu.mult,
                        )
                        nc.vector.tensor_tensor(
                            _sq(tl["ax"][:, :, 0:1]), _sq(tl["ax"][:, :, 0:1]),
                            cr[:], Alu.add,
                        )
                    dve.append(carry_ops)

                def scan_op():
                    tl["e"] = pools["e"].tile([128, NL, L], f32, tag="e",
                                              name=f"e{bi}_{rep}")
                    ef = _mk(tl["e"][:], [list(tl["e"][:].ap[0]), [1, NL * L]])
                    csf = _mk(tl["csr"][:], [list(tl["csr"][:].ap[0]), [1, NL * L]])
                    axf = _mk(tl["ax"][:], [list(tl["ax"][:].ap[0]), [1, NL * L]])
                    nc.vector.tensor_tensor_scan(
                        ef, csf, axf, 0.0, Alu.mult, Alu.add
                    )
                    # first chunk of d/q/cm on the fast path (DVE/Act) so the
                    # next chain can start while Pool produces the rest
                    tl["d"] = pools["d"].tile([128, NL, L], f32, tag="d",
                                              name=f"d{bi}_{rep}")
                    tl["q"] = pools["ax"].tile([128, NL, L], f32, tag="ax",
                                               name=f"q{bi}_{rep}")
                    tl["cm"] = pools["cm"].tile([128, NL, L], bf16, tag="cm",
                                                name=f"cm{bi}_{rep}")
                    w0 = min(CH, L)
                    nc.vector.tensor_tensor(
                        tl["d"][:, :, 0:w0], tl["e"][:, :, 0:w0],
                        tl["x"][:, :, 0:w0], Alu.subtract,
                    )
                    nc.scalar.square(tl["q"][:, :, 0:w0], tl["d"][:, :, 0:w0])
                    nc.vector.scalar_tensor_tensor(
                        tl["cm"][:, :, 0:w0], tl["q"][:, :, 0:w0], 1.0,
                        _bcast_mid(p41f_t[:, 0:w0], NL), Alu.is_ge, Alu.mult,
                    )
                    if L > w0:
                        eng("d").tensor_tensor(
                            tl["d"][:, :, w0:], tl["e"][:, :, w0:],
                            tl["x"][:, :, w0:], Alu.subtract,
                        )
                        nc.scalar.square(tl["q"][:, :, w0:], tl["d"][:, :, w0:])
                    tl["db"] = pools["db"].tile([128, NL, L], bf16, tag="db",
                                                name=f"db{bi}_{rep}")
                    nc.scalar.copy(tl["db"][:], tl["d"][:])
                dve.append(scan_op)

                def cm_op():
                    L_ = blocks[bi][1]
                    w0 = min(CH, L_)
                    if L_ > w0:
                        nc.vector.scalar_tensor_tensor(
                            tl["cm"][:, :, w0:], tl["q"][:, :, w0:], 1.0,
                            _bcast_mid(p41f_t[:, w0:L_], NL), Alu.is_ge, Alu.mult,
                        )
                    tl["m1"] = pools["m1"].tile([128, NL, L], bf16, tag="m1",
                                                name=f"m1{bi}_{rep}")
                dve.append(cm_op)
                return dve

            def emit_out(bi, rep=rep, tiles=None):
                """Emit block bi's output ops (needs its chain complete);
                returns DVE thunks."""
                t0, L = blocks[bi]
                tl = tiles[bi]
                nch_f = L // CH
                rem = L % CH
                dve = []

                def ff_op():
                    tl["ff"] = pools["ff"].tile([128, NL, L], bf16, tag="ff",
                                                name=f"ff{bi}_{rep}")
                    parts = [(0, nch_f, CH)] + (
                        [(nch_f * CH, 1, rem)] if rem else []
                    )
                    for lo, nch, w in parts:
                        ffsl = tl["ff"][:, :, lo : lo + nch * w]
                        dims = [list(x) for x in ffsl.ap]
                        st = dims[2][0]
                        ff_perm = _mk(ffsl, [dims[0], dims[1], [st, w],
                                             [st * w, nch]])
                        cl = lo // CH
                        fsl = f_blks[bi][:, :, cl : cl + nch]
                        fdims = [list(x) for x in fsl.ap]
                        f_perm = _mk(fsl, [fdims[0], fdims[1], [0, w], fdims[2]])
                        eng("ff").tensor_copy(ff_perm, f_perm)

                def m2_op():
                    tl["m2"] = pools["m2"].tile([128, NL, L], bf16, tag="m2",
                                                name=f"m2{bi}_{rep}")
                    eng("m2").tensor_tensor(
                        tl["m2"][:], _bcast_mid(p41h_t[:, :L], NL), tl["ff"][:],
                        Alu.is_gt,
                    )

                def sp_op():
                    tl["spb"] = pools["cm"].tile([128, NL, L], bf16, tag="cm",
                                                 name=f"spb{bi}_{rep}")
                    eng("sp").tensor_tensor(
                        tl["spb"][:], _bcast_mid(p41h_t[:, :L], NL), tl["ff"][:],
                        Alu.is_equal,
                    )
                    tl["sp"] = pools["sp"].tile([128, NL, L], dt.uint8,
                                                tag="sp", name=f"sp{bi}_{rep}")
                    nc.scalar.copy(tl["sp"][:], tl["spb"][:])
                    nc.gpsimd.dma_start(spv[:, bi, :, 0:L], tl["sp"][:])

                def m12_op():
                    tl["m12"] = pools["m12"].tile([128, NL, L], bf16,
                                                  tag="m12", name=f"m12{bi}_{rep}")
                    nc.vector.tensor_tensor(tl["m12"][:], tl["m1"][:],
                                            tl["m2"][:], Alu.mult)

                def va_op():
                    tl["va"] = pools["va"].tile([128, NL, L], bf16, tag="va",
                                                name=f"va{bi}_{rep}")
                    nc.vector.tensor_tensor(tl["va"][:], tl["m12"][:],
                                            tl["db"][:], Alu.mult)
                    nc.scalar.dma_start(vav[:, bi, :, 0:L], tl["va"][:])

                ff_op()  # pool op; emit immediately
                dve.extend([m2_op, sp_op, m12_op, va_op])
                return dve

            def emit_chain(bi, fillers, rep=rep, tiles=None):
                t0, L = blocks[bi]
                tl = tiles[bi]
                nch_f = L // CH
                rem = L % CH
                widths = [CH] * nch_f + ([rem] if rem else [])
                nch_b = len(widths)
                fi = 0

                def fill(n=1):
                    nonlocal fi
                    for _ in range(n):
                        if fi < len(fillers):
                            fillers[fi]()
                            fi += 1

                for ci, w in enumerate(widths):
                    cg = t0 // CH + ci
                    lo = ci * CH
                    nc.vector.tensor_tensor(
                        tl["m1"][:, :, lo : lo + w],
                        _bcast_mid(p41h_t[:, lo : lo + w], NL),
                        _col_bcast(s_blks[bi][:, :, ci : ci + 1], w),
                        Alu.is_le,
                    )
                    z_t = pools["ck"].tile([128, NL, CH], bf16, tag="Z",
                                           name=f"z{cg}_{rep}")
                    nc.vector.tensor_tensor(
                        z_t[:, :, :w], tl["m1"][:, :, lo : lo + w],
                        tl["cm"][:, :, lo : lo + w], Alu.mult,
                    )
                    nc.vector.tensor_reduce(
                        _sq(f_blks[bi][:, :, ci : ci + 1]), z_t[:, :, :w],
                        mybir.AxisListType.X, Alu.max,
                    )
                    fill()
                    if ci < nch_b - 1:
                        nxt = s_blks[bi][:, :, ci + 1 : ci + 2]
                    elif bi + 1 < nb:
                        nxt = s_blks[bi + 1][:, :, 0:1]
                    else:
                        nxt = None
                    if nxt is not None:
                        fcol = _sq(f_blks[bi][:, :, ci : ci + 1])
                        h_t = pools["ck"].tile([128, NL], bf16, tag="h",
                                               name=f"h{cg}_{rep}")
                        nc.vector.tensor_scalar(
                            h_t[:], fcol, 1.0, float(CH), Alu.is_lt, Alu.mult
                        )
                        nc.vector.tensor_tensor(_sq(nxt), fcol, h_t[:], Alu.add)
                    fill()
                fill(len(fillers))  # flush

            # software pipeline: prep(0); then for each bi: chain(bi)
            # interleaved with prep(bi+1) + out(bi-1); finally out(nb-1).
            pending = emit_prep(0, tiles=tiles)
            for f in pending:
                f()
            for bi in range(nb):
                fillers = []
                if bi + 1 < nb:
                    fillers.extend(emit_prep(bi + 1, tiles=tiles))
                if rep == 0 and bi == 0:
                    nc.sync.dma_start(
                        cs_all[:, 328:2048], cs_d.ap()[:, 328:2048]
                    )
                if rep == 0 and bi == 1:
                    nc.sync.dma_start(
                        cs_all[:, 2048:Tt], cs_d.ap()[:, 2048:Tt]
                    )
                if bi - 1 >= 0:
                    fillers.extend(emit_out(bi - 1, tiles=tiles))
                emit_chain(bi, fillers, tiles=tiles)
            for f in emit_out(nb - 1, tiles=tiles):
                f()

    nc.compile()
    return nc


def host_inputs(x_core, Tt=T):
    csr = alternating_cs(Tt).copy()
    for t0, _ in _blocks(Tt):
        csr[t0] = 0.0
    cs = np.ascontiguousarray(np.broadcast_to(csr, (128, Tt)))
    p41 = (CH - (np.arange(LB) % CH)).astype(np.float32)
    p41f = np.ascontiguousarray(np.broadcast_to(p41, (128, LB)))
    p41h = np.ascontiguousarray(p41f.astype(BF16))
    xr = np.ascontiguousarray(
        x_core.reshape(BL, Tt, G, 128).transpose(3, 0, 2, 1), np.float32
    ).reshape(128, NL, Tt)
    nb = len(_blocks(Tt))
    xp = np.zeros((128, nb, NL, LB), np.float32)
    for bi, (t0, L) in enumerate(_blocks(Tt)):
        xp[:, bi, :, 0:L] = xr[:, :, t0 : t0 + L]
    return {"x": xp, "cs": cs, "p41f": p41f, "p41h": p41h}


def _assemble(blk, Tt, np_dtype, off):
    """[128, nb, NL, LB] block-major -> [128, NL, Tt+1] linear at offset."""
    out = np.zeros((128, NL, Tt + 1), np_dtype)
    for bi, (t0, L) in enumerate(_blocks(Tt)):
        out[:, :, off + t0 : off + t0 + L] = blk[:, bi, :, 0:L]
    return out


_NC = None
LAST_EXEC_NS = None
LAST_RESULT = None


def kernel(input_current, vb_t=None, A_t=None, th_t=None, gain_t=None, tref_t=None):
    global _NC, LAST_EXEC_NS, LAST_RESULT
    x = np.ascontiguousarray(np.asarray(input_current), np.float32)
    assert x.shape == (B, T, F), x.shape
    if _NC is None:
        _NC = build(T)
    in_maps = [host_inputs(x[k * BL : (k + 1) * BL]) for k in range(NCORES)]
    res = run_bass_kernel_spmd(_NC, in_maps, core_ids=list(range(NCORES)))
    LAST_EXEC_NS = res.exec_time_ns
    LAST_RESULT = res

    def untr(a3):
        p, nl, tt = a3.shape
        return a3.reshape(p, BL, G, tt).transpose(1, 3, 2, 0).reshape(
            BL, tt, G * p
        )

    vas, sps = [], []
    for k in range(NCORES):
        vab = _assemble(res.results[k]["va"].astype(np.float32), T, np.float32, 1)
        vab[:, :, 0] = 0.0
        spb = _assemble(res.results[k]["sp"], T, np.uint8, 0)
        spb[:, :, T] = spb[:, :, T - 1]
        vas.append(untr(vab))
        sps.append(untr(spb))
    va = np.concatenate(vas, axis=0)
    sp = np.concatenate(sps, axis=0)
    return va, sp.astype(bool)
